# revision 7
# baseline (speedup 1.0000x reference)
"""Local/global multihead attention on 8 NeuronCores (Trainium2, Bass/Tile).

Sharding: core c = b*2 + hg  (b = batch 0..3, hg = head-group 0/1, 8 heads each).
Per core: q/k/v projections for its 8 heads, head-local attention (slot 0 runs a
dense 2048-key path driven by per-core gmask data: hg0's slot 0 is the true
global head with an all-ones mask, hg1's slot 0 is a local head with a band
mask), banded attention for slots 1-7, out-projection restricted to the
head-group's wo columns.  Host sums the two head-group partials per batch and
adds bo + bv @ wo.T (softmax rows sum to 1).

Key perf structure vs the naive version:
- s-outer attention loop: dense(s) -> local pairs(s) -> out-proj(s-1)
  interleaved, so PE/ACT/DVE pipeline and the PE never idles long (HAM warm).
- Banded scores for one (head, s) packed into ONE 3-bank PSUM tile [128,1536]
  (window widths 128+256+384+384+256+128), so exp is a single ACT instruction
  and the band-mask multiply is a single DVE instruction.
- Softmax denominator comes from a 65th ones-column in v (matmul-accumulated),
  inverted with reciprocal_approx_fast (~5x faster than InstReciprocal),
  broadcast on GpSimd -- the whole normalize chain is off the critical path.
- Local QK matmuls of even/odd head pairs are emitted adjacently: K=64 row
  tiles at base partitions 0/64 co-execute in the PE array.
- Projections run tcn-outer so each PSUM group is 8 back-to-back matmuls;
  evictions (bias add) on DVE overlap.

All matmul operands bf16 (1 cyc/col); PSUM accumulation fp32.
"""
import numpy as np
import ml_dtypes

E, H, D, LK = 1024, 16, 64, 128
SCALE = D ** -0.5
B, N = 4, 2048
FG = 512          # features per head-group (8 heads * 64)
NCORES = 8

# banded-window geometry per dj (delta = (dj-1)*128): query-col range in the
# 512-chunk, packed offset in the [128,1536] score tile, and PSUM-bank-safe
# matmul splits (packed_lo, packed_hi, q_lo, q_hi)
WIN = [(0, 128), (0, 256), (0, 384), (128, 512), (256, 512), (384, 512)]
OFF = [0, 128, 384, 768, 1152, 1408]
SPLIT = {
    0: [(0, 128, 0, 128)],
    1: [(128, 384, 0, 256)],
    2: [(384, 512, 0, 128), (512, 768, 128, 384)],
    3: [(768, 1024, 128, 384), (1024, 1152, 384, 512)],
    4: [(1152, 1408, 256, 512)],
    5: [(1408, 1536, 384, 512)],
}

_cache = {}


def _bf16(a):
    return np.ascontiguousarray(a.astype(ml_dtypes.bfloat16))


def _build():
    import concourse.bacc as bacc
    import concourse.tile as tile
    import concourse.mybir as mybir
    from concourse.bass import ts

    dt = mybir.dt
    AF = mybir.ActivationFunctionType

    nc = bacc.Bacc("TRN2", target_bir_lowering=False, debug=False,
                   num_devices=NCORES)

    xT = nc.dram_tensor("xT", [E, N], dt.bfloat16, kind="ExternalInput")
    wqT = nc.dram_tensor("wqT", [E, FG], dt.bfloat16, kind="ExternalInput")
    wkT = nc.dram_tensor("wkT", [E, FG], dt.bfloat16, kind="ExternalInput")
    wvT = nc.dram_tensor("wvT", [E, FG], dt.bfloat16, kind="ExternalInput")
    woT = nc.dram_tensor("woT", [FG, E], dt.bfloat16, kind="ExternalInput")
    bqc = nc.dram_tensor("bqc", [128, 4], dt.float32, kind="ExternalInput")
    bkc = nc.dram_tensor("bkc", [128, 4], dt.float32, kind="ExternalInput")
    lmask = nc.dram_tensor("lmask", [128, 1536], dt.bfloat16, kind="ExternalInput")
    gmask = nc.dram_tensor("gmask", [4, 8, 128, 1024], dt.bfloat16, kind="ExternalInput")
    out = nc.dram_tensor("out", [N, E], dt.float32, kind="ExternalOutput")

    with tile.TileContext(nc) as tc:
        with (
            tc.tile_pool(name="wts", bufs=1) as wts,
            tc.tile_pool(name="xp", bufs=1) as xp,
            tc.tile_pool(name="qkv", bufs=1) as qkv,
            tc.tile_pool(name="att", bufs=3) as attp,
            tc.tile_pool(name="gm", bufs=3) as gmp,
            tc.tile_pool(name="small", bufs=4) as small,
            tc.tile_pool(name="ob", bufs=2) as obp,
            tc.tile_pool(name="sc", bufs=2, space="PSUM") as scp,
            tc.tile_pool(name="avb", bufs=2, space="PSUM") as avp,
        ):
            # ---- input DMAs, ordered so the k-projection can start earliest
            wk_t = [wts.tile([128, FG], dt.bfloat16, name=f"wk{i}", tag=f"wk{i}") for i in range(8)]
            for ec in range(8):
                nc.sync.dma_start(wk_t[ec][:], wkT[ts(ec, 128), :])
            xT_t = [xp.tile([128, N], dt.bfloat16, name=f"xT{i}", tag=f"xT{i}") for i in range(8)]
            for ec in range(8):  # two DMAs per tile: spread across more queues
                nc.sync.dma_start(xT_t[ec][:, 0:1024], xT[ts(ec, 128), 0:1024])
                nc.sync.dma_start(xT_t[ec][:, 1024:2048], xT[ts(ec, 128), 1024:2048])
            bk_t = small.tile([128, 4], dt.float32, name="bk", tag="bk", bufs=1)
            nc.sync.dma_start(bk_t[:], bkc[:, :])
            wq_t = [wts.tile([128, FG], dt.bfloat16, name=f"wq{i}", tag=f"wq{i}") for i in range(8)]
            for ec in range(8):
                nc.sync.dma_start(wq_t[ec][:], wqT[ts(ec, 128), :])
            bq_t = small.tile([128, 4], dt.float32, name="bq", tag="bq", bufs=1)
            nc.sync.dma_start(bq_t[:], bqc[:, :])
            wv_t = [wts.tile([128, FG], dt.bfloat16, name=f"wv{i}", tag=f"wv{i}") for i in range(8)]
            for ec in range(8):
                nc.sync.dma_start(wv_t[ec][:], wvT[ts(ec, 128), :])
            wo_t = [wts.tile([128, E], dt.bfloat16, name=f"wo{i}", tag=f"wo{i}") for i in range(4)]
            for fc in range(4):
                nc.sync.dma_start(wo_t[fc][:], woT[ts(fc, 128), :])
            lm_t = wts.tile([128, 1536], dt.bfloat16, name="lm", tag="lm")
            nc.sync.dma_start(lm_t[:], lmask[:, :])

            # preload the exp table set while projections run
            warm = small.tile([128, 4], dt.float32, name="warm", tag="warm", bufs=1)
            nc.scalar.activation(warm[:], bk_t[:], AF.Exp)

            # ---- projections (tcn-outer: one PSUM group = 8 dense matmuls)
            kT_sb = [qkv.tile([128, N], dt.bfloat16, name=f"kT{i}", tag=f"kT{i}") for i in range(4)]
            qT_sb = [qkv.tile([128, N], dt.bfloat16, name=f"qT{i}", tag=f"qT{i}") for i in range(4)]
            for dst, w_t, b_t in ((kT_sb, wk_t, bk_t), (qT_sb, wq_t, bq_t)):
                for tcn in range(4):
                    for fc in range(4):
                        ps = avp.tile([128, 512], dt.float32, name="ps", tag="av")
                        for ec in range(8):
                            nc.tensor.matmul(
                                ps[:], w_t[ec][:, ts(fc, 128)],
                                xT_t[ec][:, ts(tcn, 512)],
                                start=(ec == 0), stop=(ec == 7))
                        nc.vector.tensor_scalar_add(
                            dst[fc][:, ts(tcn, 512)], ps[:], b_t[:, fc:fc + 1])
            # v natural layout, per-head 72-col strided tiles with ones col
            v_sb = [qkv.tile([128, 8 * 72], dt.bfloat16, name=f"v{i}", tag=f"v{i}") for i in range(16)]
            for tcn in range(16):
                ps = avp.tile([128, 512], dt.float32, name="ps", tag="av")
                for ec in range(8):
                    nc.tensor.matmul(ps[:], xT_t[ec][:, ts(tcn, 128)], wv_t[ec][:],
                                     start=(ec == 0), stop=(ec == 7))
                src = ps[:].rearrange("p (h d) -> p h d", h=8)
                dst = v_sb[tcn][:].rearrange("p (h d) -> p h d", h=8)[:, :, 0:64]
                nc.vector.tensor_copy(dst, src)
                ones = v_sb[tcn][:].rearrange("p (h d) -> p h d", h=8)[:, :, 64:65]
                nc.vector.memset(ones, 1.0)

            outTn = [qkv.tile([128, N], dt.bfloat16, name=f"outTn{i}", tag=f"outTn{i}") for i in range(4)]

            def head_rows(t, h):
                r0 = (h % 2) * 64
                return t[h // 2][r0:r0 + 64, :]

            # softmax denominators: per head, copy the ones-row out of PSUM,
            # DMA-gather all 8 of the chunk into one [8,512] tile, invert with
            # ONE InstReciprocal (cost is free-size-driven: 3.3us for all 8
            # heads instead of per-head), then scatter + broadcast + multiply.
            def evict_phase1(av, h, pack, nums):
                den1 = small.tile([1, 512], dt.float32, name="den1", tag="den1", bufs=4)
                nc.vector.tensor_copy(den1[:], av[64:65, :])
                nc.sync.dma_start(pack[h:h + 1, :], den1[:])
                num = small.tile([64, 512], dt.bfloat16, name="num", tag="num", bufs=9)
                nc.vector.tensor_copy(num[:], av[0:64, :])
                nums[h] = num

            def evict_phase2(s, pack, nums):
                rpak = small.tile([8, 512], dt.float32, name="rpak", tag="rpak", bufs=2)
                nc.vector.reciprocal(rpak[:], pack[:])
                for h in range(8):
                    rec = small.tile([1, 512], dt.float32, name="rec", tag="rec", bufs=4)
                    nc.sync.dma_start(rec[:], rpak[h:h + 1, :])
                    rec64 = small.tile([64, 512], dt.float32, name="rec64", tag="rec64", bufs=4)
                    nc.gpsimd.partition_broadcast(rec64[:], rec[:])
                    nc.vector.tensor_mul(head_rows(outTn, h)[:, ts(s, 512)],
                                         nums[h][:], rec64[:])

            def op_group(s, grp):
                # 2 of the 8 (tcn, oc) out-projection psum groups of chunk s
                for k in range(2):
                    idx = grp * 2 + k
                    tcn, oc = s * 4 + idx // 2, idx % 2
                    ps = avp.tile([128, 512], dt.float32, name="ps", tag="av")
                    for fc in range(4):
                        nc.tensor.matmul(ps[:], outTn[fc][:, ts(tcn, 128)],
                                         wo_t[fc][:, ts(oc, 512)],
                                         start=(fc == 0), stop=(fc == 3))
                    obt = obp.tile([128, 512], dt.float32, name="obt", tag="ob")
                    nc.scalar.copy(obt[:], ps[:])
                    nc.sync.dma_start(out[ts(tcn, 128), ts(oc, 512)], obt[:])

            # ---- attention: s-outer; dense slot0, then local head pairs with
            # the previous chunk's out-projection interleaved
            pairs = [(2, 1), (4, 3), (6, 5), (7,)]
            qh0, kh0 = head_rows(qT_sb, 0), head_rows(kT_sb, 0)
            for s in range(4):
                pack = small.tile([8, 512], dt.float32, name="pack", tag="pack", bufs=2)
                nums = {}
                av_d = avp.tile([128, 512], dt.float32, name="avd", tag="av")
                for jg in range(8):
                    gt = gmp.tile([128, 1024], dt.bfloat16, name="gt", tag="gm")
                    nc.sync.dma_start(gt[:], gmask[s, jg, :, :])
                    sc = scp.tile([128, 1536], dt.float32, name="sc", tag="sc")
                    at = attp.tile([128, 1536], dt.bfloat16, name="at", tag="at")
                    for j in range(2):
                        nc.tensor.matmul(
                            sc[:, ts(j, 512)], kh0[:, ts(2 * jg + j, 128)],
                            qh0[:, ts(s, 512)], start=True, stop=True,
                            skip_group_check=True)
                    nc.scalar.activation(at[:, 0:1024], sc[:, 0:1024], AF.Exp,
                                         scale=float(SCALE))
                    nc.gpsimd.tensor_mul(at[:, 0:1024], at[:, 0:1024], gt[:])
                    for j in range(2):
                        nc.tensor.matmul(
                            av_d[0:65, :], v_sb[2 * jg + j][:, 0:65],
                            at[:, ts(j, 512)],
                            start=(jg == 0 and j == 0),
                            stop=(jg == 7 and j == 1), skip_group_check=True)
                evict_phase1(av_d, 0, pack, nums)

                djs = [dj for dj in range(6) if 0 <= s * 4 - 1 + dj <= 15]
                lo = OFF[djs[0]]
                hi = OFF[djs[-1]] + WIN[djs[-1]][1] - WIN[djs[-1]][0]
                pieces = [(dj, pc) for dj in djs for pc in SPLIT[dj]]
                bank_last = {}
                for i, (dj, (plo, phi, qlo, qhi)) in enumerate(pieces):
                    bank_last[plo // 512] = i
                for ri, pr in enumerate(pairs):
                    scs = {h: scp.tile([128, 1536], dt.float32, name=f"sc{h}", tag="sc")
                           for h in pr}
                    started = {h: set() for h in pr}
                    for i, (dj, (plo, phi, qlo, qhi)) in enumerate(pieces):
                        jc = s * 4 - 1 + dj
                        for h in pr:  # even head rows 0:64, odd rows 64:128 ->
                            st = plo // 512 not in started[h]  # co-execute
                            started[h].add(plo // 512)
                            nc.tensor.matmul(
                                scs[h][:, plo:phi],
                                head_rows(kT_sb, h)[:, ts(jc, 128)],
                                head_rows(qT_sb, h)[:, s * 512 + qlo:s * 512 + qhi],
                                start=st, stop=(bank_last[plo // 512] == i),
                                skip_group_check=True)
                    ats = {}
                    for h in pr:
                        ats[h] = attp.tile([128, 1536], dt.bfloat16, name=f"at{h}", tag="at")
                        nc.scalar.activation(ats[h][:, lo:hi], scs[h][:, lo:hi],
                                             AF.Exp, scale=float(SCALE))
                    for h in pr:
                        nc.gpsimd.tensor_mul(ats[h][:, lo:hi], ats[h][:, lo:hi],
                                             lm_t[:, lo:hi])
                    for h in pr:
                        av = avp.tile([128, 512], dt.float32, name="av", tag="av")
                        for i2, dj in enumerate(djs):
                            jc = s * 4 - 1 + dj
                            c0, c1 = WIN[dj]
                            nc.tensor.matmul(
                                av[0:65, c0:c1], v_sb[jc][:, h * 72:h * 72 + 65],
                                ats[h][:, OFF[dj]:OFF[dj] + c1 - c0],
                                start=(i2 == 0), stop=(i2 == len(djs) - 1),
                                skip_group_check=True)
                        evict_phase1(av, h, pack, nums)
                    if s > 0:
                        op_group(s - 1, ri)
                evict_phase2(s, pack, nums)
            for g in range(4):
                op_group(3, g)
    nc.finalize()
    return nc


def _host_inputs(x, wq, bq, wk, bk, wv, bv, wo, bo):
    """Build the 8 per-core input dicts."""
    r = np.arange(128)[:, None]

    # packed band mask [128, 1536]: window dj at cols OFF[dj], value at
    # (r, c') = |(dj-1)*128 + r - (c0(dj) + c')| <= LK
    lm = np.zeros((128, 1536), np.float32)
    for dj in range(6):
        c0, c1 = WIN[dj]
        cols = np.arange(c0, c1)[None, :]
        lm[:, OFF[dj]:OFF[dj] + c1 - c0] = (np.abs((dj - 1) * 128 + r - cols) <= LK)
    lm = _bf16(lm)

    # dense mask [4 s, 8 jg, 128, 2*512]: cols j*512+c for key chunk 2jg+j,
    # queries 512s+c
    gm_band = np.zeros((4, 8, 128, 1024), np.float32)
    for s in range(4):
        for jg in range(8):
            for j in range(2):
                jc = 2 * jg + j
                c = np.arange(512)[None, :]
                gm_band[s, jg, :, j * 512:(j + 1) * 512] = (
                    np.abs(128 * jc + r - (512 * s + c)) <= LK)
    gm_ones = _bf16(np.ones((4, 8, 128, 1024), np.float32))
    gm_band = _bf16(gm_band)

    in_maps = []
    for core in range(NCORES):
        b, hg = core // 2, core % 2
        fsl = slice(hg * FG, (hg + 1) * FG)
        in_maps.append({
            "xT": _bf16(x[b].T),
            "wqT": _bf16(wq[fsl].T),
            "wkT": _bf16(wk[fsl].T),
            "wvT": _bf16(wv[fsl].T),
            "woT": _bf16(wo[:, fsl].T),
            "bqc": np.ascontiguousarray(bq[fsl].reshape(4, 128).T, np.float32),
            "bkc": np.ascontiguousarray(bk[fsl].reshape(4, 128).T, np.float32),
            "lmask": lm,
            "gmask": gm_ones if hg == 0 else gm_band,
        })
    return in_maps


def kernel(x, wq, bq, wk, bk, wv, bv, wo, bo):
    from concourse.bass_utils import run_bass_kernel_spmd

    x, wq, bq, wk, bk, wv, bv, wo, bo = (
        np.asarray(a, np.float32) for a in (x, wq, bq, wk, bk, wv, bv, wo, bo))

    if "nc" not in _cache:
        _cache["nc"] = _build()
    nc = _cache["nc"]

    in_maps = _host_inputs(x, wq, bq, wk, bk, wv, bv, wo, bo)
    res = run_bass_kernel_spmd(nc, in_maps, core_ids=list(range(NCORES)))
    _cache["last_results"] = res

    const = (bo + bv @ wo.T).astype(np.float32)        # [1024]
    out = np.empty((B, N, E), np.float32)
    for b in range(B):
        out[b] = res.results[2 * b]["out"] + res.results[2 * b + 1]["out"] + const
    return out


# revision 10
# speedup vs baseline: 1.4359x; 1.4359x over previous
"""Local/global multihead attention on 8 NeuronCores (Trainium2, Bass/Tile).

Sharding: core c = b*2 + hg  (b = batch 0..3, hg = head-group 0/1, 8 heads each).
Per core: q/k/v projections for its 8 heads, head-local attention (slot 0 runs a
dense 2048-key path driven by per-core gmask data: hg0's slot 0 is the true
global head with an all-ones mask, hg1's slot 0 is a local head with a band
mask), banded attention for slots 1-7, out-projection restricted to the
head-group's wo columns.  Host sums the two head-group partials per batch and
adds bo + bv @ wo.T (softmax rows sum to 1).

Key perf structure vs the naive version:
- s-outer attention loop: dense(s) -> local pairs(s) -> out-proj(s-1)
  interleaved, so PE/ACT/DVE pipeline and the PE never idles long (HAM warm).
- Banded scores for one (head, s) packed into ONE 3-bank PSUM tile [128,1536]
  (window widths 128+256+384+384+256+128), so exp is a single ACT instruction
  and the band-mask multiply is a single DVE instruction.
- Softmax denominator comes from a 65th ones-column in v (matmul-accumulated),
  inverted with reciprocal_approx_fast (~5x faster than InstReciprocal),
  broadcast on GpSimd -- the whole normalize chain is off the critical path.
- Local QK matmuls of even/odd head pairs are emitted adjacently: K=64 row
  tiles at base partitions 0/64 co-execute in the PE array.
- Projections run tcn-outer so each PSUM group is 8 back-to-back matmuls;
  evictions (bias add) on DVE overlap.

All matmul operands bf16 (1 cyc/col); PSUM accumulation fp32.
"""
import numpy as np
import ml_dtypes

E, H, D, LK = 1024, 16, 64, 128
SCALE = D ** -0.5
B, N = 4, 2048
FG = 512          # features per head-group (8 heads * 64)
NCORES = 8

# banded-window geometry per dj (delta = (dj-1)*128): query-col range in the
# 512-chunk, packed offset in the [128,1536] score tile, and PSUM-bank-safe
# matmul splits (packed_lo, packed_hi, q_lo, q_hi)
WIN = [(0, 128), (0, 256), (0, 384), (128, 512), (256, 512), (384, 512)]
OFF = [0, 128, 384, 768, 1152, 1408]
SPLIT = {
    0: [(0, 128, 0, 128)],
    1: [(128, 384, 0, 256)],
    2: [(384, 512, 0, 128), (512, 768, 128, 384)],
    3: [(768, 1024, 128, 384), (1024, 1152, 384, 512)],
    4: [(1152, 1408, 256, 512)],
    5: [(1408, 1536, 384, 512)],
}

_cache = {}


def _bf16(a):
    return np.ascontiguousarray(a.astype(ml_dtypes.bfloat16))


def _build():
    import concourse.bacc as bacc
    import concourse.tile as tile
    import concourse.mybir as mybir
    from concourse.bass import ts

    dt = mybir.dt
    AF = mybir.ActivationFunctionType

    nc = bacc.Bacc("TRN2", target_bir_lowering=False, debug=False,
                   num_devices=NCORES)

    xT = nc.dram_tensor("xT", [E, N], dt.bfloat16, kind="ExternalInput")
    wqT = nc.dram_tensor("wqT", [E, FG], dt.bfloat16, kind="ExternalInput")
    wkT = nc.dram_tensor("wkT", [E, FG], dt.bfloat16, kind="ExternalInput")
    wvT = nc.dram_tensor("wvT", [E, FG], dt.bfloat16, kind="ExternalInput")
    woT = nc.dram_tensor("woT", [FG, E], dt.bfloat16, kind="ExternalInput")
    bqc = nc.dram_tensor("bqc", [128, 4], dt.float32, kind="ExternalInput")
    bkc = nc.dram_tensor("bkc", [128, 4], dt.float32, kind="ExternalInput")
    lmask = nc.dram_tensor("lmask", [128, 1536], dt.bfloat16, kind="ExternalInput")
    gmask = nc.dram_tensor("gmask", [4, 8, 128, 1024], dt.bfloat16, kind="ExternalInput")
    out = nc.dram_tensor("out", [N, E], dt.float32, kind="ExternalOutput")

    with tile.TileContext(nc) as tc:
        with (
            tc.tile_pool(name="wts", bufs=1) as wts,
            tc.tile_pool(name="xp", bufs=1) as xp,
            tc.tile_pool(name="qkv", bufs=1) as qkv,
            tc.tile_pool(name="att", bufs=3) as attp,
            tc.tile_pool(name="gm", bufs=3) as gmp,
            tc.tile_pool(name="small", bufs=4) as small,
            tc.tile_pool(name="ob", bufs=2) as obp,
            tc.tile_pool(name="sc", bufs=2, space="PSUM") as scp,
            tc.tile_pool(name="avb", bufs=2, space="PSUM") as avp,
        ):
            # ---- input DMAs, ordered so the k-projection can start earliest
            wk_t = [wts.tile([128, FG], dt.bfloat16, name=f"wk{i}", tag=f"wk{i}") for i in range(8)]
            for ec in range(8):
                nc.sync.dma_start(wk_t[ec][:], wkT[ts(ec, 128), :])
            xT_t = [xp.tile([128, N], dt.bfloat16, name=f"xT{i}", tag=f"xT{i}") for i in range(8)]
            for half in range(2):  # first halves first: k-proj tcn0/1 unblock early
                for ec in range(8):
                    nc.sync.dma_start(xT_t[ec][:, ts(half, 1024)],
                                      xT[ts(ec, 128), ts(half, 1024)])
            bk_t = small.tile([128, 4], dt.float32, name="bk", tag="bk", bufs=1)
            nc.sync.dma_start(bk_t[:], bkc[:, :])
            wq_t = [wts.tile([128, FG], dt.bfloat16, name=f"wq{i}", tag=f"wq{i}") for i in range(8)]
            for ec in range(8):
                nc.sync.dma_start(wq_t[ec][:], wqT[ts(ec, 128), :])
            bq_t = small.tile([128, 4], dt.float32, name="bq", tag="bq", bufs=1)
            nc.sync.dma_start(bq_t[:], bqc[:, :])
            wv_t = [wts.tile([128, FG], dt.bfloat16, name=f"wv{i}", tag=f"wv{i}") for i in range(8)]
            for ec in range(8):
                nc.sync.dma_start(wv_t[ec][:], wvT[ts(ec, 128), :])
            wo_t = [wts.tile([128, E], dt.bfloat16, name=f"wo{i}", tag=f"wo{i}") for i in range(4)]
            for fc in range(4):
                nc.sync.dma_start(wo_t[fc][:], woT[ts(fc, 128), :])
            lm_t = wts.tile([128, 1536], dt.bfloat16, name="lm", tag="lm")
            nc.sync.dma_start(lm_t[:], lmask[:, :])

            # preload the exp table set while projections run
            warm = small.tile([128, 4], dt.float32, name="warm", tag="warm", bufs=1)
            nc.scalar.activation(warm[:], bk_t[:], AF.Exp)

            # ---- projections (tcn-outer: one PSUM group = 8 dense matmuls)
            kT_sb = [qkv.tile([128, N], dt.bfloat16, name=f"kT{i}", tag=f"kT{i}") for i in range(4)]
            qT_sb = [qkv.tile([128, N], dt.bfloat16, name=f"qT{i}", tag=f"qT{i}") for i in range(4)]
            for dst, w_t, b_t in ((kT_sb, wk_t, bk_t), (qT_sb, wq_t, bq_t)):
                for tcn in range(4):
                    for fc in range(4):
                        ps = avp.tile([128, 512], dt.float32, name="ps", tag="av")
                        for ec in range(8):
                            nc.tensor.matmul(
                                ps[:], w_t[ec][:, ts(fc, 128)],
                                xT_t[ec][:, ts(tcn, 512)],
                                start=(ec == 0), stop=(ec == 7))
                        nc.vector.tensor_scalar_add(
                            dst[fc][:, ts(tcn, 512)], ps[:], b_t[:, fc:fc + 1])
            # v natural layout, per-head 72-col strided tiles with ones col
            v_sb = [qkv.tile([128, 8 * 72], dt.bfloat16, name=f"v{i}", tag=f"v{i}") for i in range(16)]
            for tcn in range(16):
                ps = avp.tile([128, 512], dt.float32, name="ps", tag="av")
                for ec in range(8):
                    nc.tensor.matmul(ps[:], xT_t[ec][:, ts(tcn, 128)], wv_t[ec][:],
                                     start=(ec == 0), stop=(ec == 7))
                src = ps[:].rearrange("p (h d) -> p h d", h=8)
                dst = v_sb[tcn][:].rearrange("p (h d) -> p h d", h=8)[:, :, 0:64]
                nc.vector.tensor_copy(dst, src)
                ones = v_sb[tcn][:].rearrange("p (h d) -> p h d", h=8)[:, :, 64:65]
                nc.vector.memset(ones, 1.0)

            outTn = [qkv.tile([128, N], dt.bfloat16, name=f"outTn{i}", tag=f"outTn{i}") for i in range(4)]

            def head_rows(t, h):
                r0 = (h % 2) * 64
                return t[h // 2][r0:r0 + 64, :]

            # softmax denominators: per head, copy the ones-row out of PSUM,
            # DMA-gather all 8 of the chunk into one [8,512] tile, invert with
            # ONE InstReciprocal (cost is free-size-driven: 3.3us for all 8
            # heads instead of per-head), then scatter + broadcast + multiply.
            def evict_phase1(av, h, pack, nums):
                den1 = small.tile([1, 512], dt.float32, name="den1", tag="den1", bufs=4)
                nc.vector.tensor_copy(den1[:], av[64:65, :])
                nc.sync.dma_start(pack[h:h + 1, :], den1[:])
                num = small.tile([64, 512], dt.bfloat16, name="num", tag="num", bufs=9)
                nc.vector.tensor_copy(num[:], av[0:64, :])
                nums[h] = num

            def evict_phase2(s, pack, nums):
                rpak = small.tile([8, 512], dt.float32, name="rpak", tag="rpak", bufs=2)
                nc.vector.reciprocal(rpak[:], pack[:])
                for h in range(8):
                    rec = small.tile([1, 512], dt.float32, name="rec", tag="rec", bufs=4)
                    nc.sync.dma_start(rec[:], rpak[h:h + 1, :])
                    rec64 = small.tile([64, 512], dt.float32, name="rec64", tag="rec64", bufs=4)
                    nc.gpsimd.partition_broadcast(rec64[:], rec[:])
                    nc.vector.tensor_mul(head_rows(outTn, h)[:, ts(s, 512)],
                                         nums[h][:], rec64[:])

            def op_group(s, grp):
                # 2 of the 8 (tcn, oc) out-projection psum groups of chunk s
                for k in range(2):
                    idx = grp * 2 + k
                    tcn, oc = s * 4 + idx // 2, idx % 2
                    ps = avp.tile([128, 512], dt.float32, name="ps", tag="av")
                    for fc in range(4):
                        nc.tensor.matmul(ps[:], outTn[fc][:, ts(tcn, 128)],
                                         wo_t[fc][:, ts(oc, 512)],
                                         start=(fc == 0), stop=(fc == 3))
                    obt = obp.tile([128, 512], dt.float32, name="obt", tag="ob")
                    nc.scalar.copy(obt[:], ps[:])
                    nc.sync.dma_start(out[ts(tcn, 128), ts(oc, 512)], obt[:])

            # ---- attention: s-outer; dense slot0, then local head pairs with
            # the previous chunk's out-projection interleaved
            pairs = [(2, 1), (4, 3), (6, 5), (7,)]
            qh0, kh0 = head_rows(qT_sb, 0), head_rows(kT_sb, 0)
            for s in range(4):
                pack = small.tile([8, 512], dt.float32, name="pack", tag="pack", bufs=2)
                nums = {}
                av_d = avp.tile([128, 512], dt.float32, name="avd", tag="av")
                for jg in range(8):
                    gt = gmp.tile([128, 1024], dt.bfloat16, name="gt", tag="gm")
                    nc.sync.dma_start(gt[:], gmask[s, jg, :, :])
                    sc = scp.tile([128, 1536], dt.float32, name="sc", tag="sc")
                    at = attp.tile([128, 1536], dt.bfloat16, name="at", tag="at")
                    for j in range(2):
                        nc.tensor.matmul(
                            sc[:, ts(j, 512)], kh0[:, ts(2 * jg + j, 128)],
                            qh0[:, ts(s, 512)], start=True, stop=True,
                            skip_group_check=True)
                    nc.scalar.activation(at[:, 0:1024], sc[:, 0:1024], AF.Exp,
                                         scale=float(SCALE))
                    nc.vector.tensor_mul(at[:, 0:1024], at[:, 0:1024], gt[:])
                    for j in range(2):
                        nc.tensor.matmul(
                            av_d[0:65, :], v_sb[2 * jg + j][:, 0:65],
                            at[:, ts(j, 512)],
                            start=(jg == 0 and j == 0),
                            stop=(jg == 7 and j == 1), skip_group_check=True)
                evict_phase1(av_d, 0, pack, nums)

                djs = [dj for dj in range(6) if 0 <= s * 4 - 1 + dj <= 15]
                lo = OFF[djs[0]]
                hi = OFF[djs[-1]] + WIN[djs[-1]][1] - WIN[djs[-1]][0]
                pieces = [(dj, pc) for dj in djs for pc in SPLIT[dj]]
                bank_last = {}
                for i, (dj, (plo, phi, qlo, qhi)) in enumerate(pieces):
                    bank_last[plo // 512] = i
                for ri, pr in enumerate(pairs):
                    scs = {h: scp.tile([128, 1536], dt.float32, name=f"sc{h}", tag="sc")
                           for h in pr}
                    started = {h: set() for h in pr}
                    for i, (dj, (plo, phi, qlo, qhi)) in enumerate(pieces):
                        jc = s * 4 - 1 + dj
                        for h in pr:  # even head rows 0:64, odd rows 64:128 ->
                            st = plo // 512 not in started[h]  # co-execute
                            started[h].add(plo // 512)
                            nc.tensor.matmul(
                                scs[h][:, plo:phi],
                                head_rows(kT_sb, h)[:, ts(jc, 128)],
                                head_rows(qT_sb, h)[:, s * 512 + qlo:s * 512 + qhi],
                                start=st, stop=(bank_last[plo // 512] == i),
                                skip_group_check=True)
                    ats = {}
                    for h in pr:
                        ats[h] = attp.tile([128, 1536], dt.bfloat16, name=f"at{h}", tag="at")
                        nc.scalar.activation(ats[h][:, lo:hi], scs[h][:, lo:hi],
                                             AF.Exp, scale=float(SCALE))
                    for h in pr:
                        nc.vector.tensor_mul(ats[h][:, lo:hi], ats[h][:, lo:hi],
                                             lm_t[:, lo:hi])
                    for h in pr:
                        av = avp.tile([128, 512], dt.float32, name="av", tag="av")
                        for i2, dj in enumerate(djs):
                            jc = s * 4 - 1 + dj
                            c0, c1 = WIN[dj]
                            nc.tensor.matmul(
                                av[0:65, c0:c1], v_sb[jc][:, h * 72:h * 72 + 65],
                                ats[h][:, OFF[dj]:OFF[dj] + c1 - c0],
                                start=(i2 == 0), stop=(i2 == len(djs) - 1),
                                skip_group_check=True)
                        evict_phase1(av, h, pack, nums)
                    if s > 0:
                        op_group(s - 1, ri)
                evict_phase2(s, pack, nums)
            for g in range(4):
                op_group(3, g)
    nc.finalize()
    return nc


def _host_inputs(x, wq, bq, wk, bk, wv, bv, wo, bo):
    """Build the 8 per-core input dicts."""
    r = np.arange(128)[:, None]

    # packed band mask [128, 1536]: window dj at cols OFF[dj], value at
    # (r, c') = |(dj-1)*128 + r - (c0(dj) + c')| <= LK
    lm = np.zeros((128, 1536), np.float32)
    for dj in range(6):
        c0, c1 = WIN[dj]
        cols = np.arange(c0, c1)[None, :]
        lm[:, OFF[dj]:OFF[dj] + c1 - c0] = (np.abs((dj - 1) * 128 + r - cols) <= LK)
    lm = _bf16(lm)

    # dense mask [4 s, 8 jg, 128, 2*512]: cols j*512+c for key chunk 2jg+j,
    # queries 512s+c
    gm_band = np.zeros((4, 8, 128, 1024), np.float32)
    for s in range(4):
        for jg in range(8):
            for j in range(2):
                jc = 2 * jg + j
                c = np.arange(512)[None, :]
                gm_band[s, jg, :, j * 512:(j + 1) * 512] = (
                    np.abs(128 * jc + r - (512 * s + c)) <= LK)
    gm_ones = _bf16(np.ones((4, 8, 128, 1024), np.float32))
    gm_band = _bf16(gm_band)

    in_maps = []
    for core in range(NCORES):
        b, hg = core // 2, core % 2
        fsl = slice(hg * FG, (hg + 1) * FG)
        in_maps.append({
            "xT": _bf16(x[b].T),
            "wqT": _bf16(wq[fsl].T),
            "wkT": _bf16(wk[fsl].T),
            "wvT": _bf16(wv[fsl].T),
            "woT": _bf16(wo[:, fsl].T),
            "bqc": np.ascontiguousarray(bq[fsl].reshape(4, 128).T, np.float32),
            "bkc": np.ascontiguousarray(bk[fsl].reshape(4, 128).T, np.float32),
            "lmask": lm,
            "gmask": gm_ones if hg == 0 else gm_band,
        })
    return in_maps


def kernel(x, wq, bq, wk, bk, wv, bv, wo, bo):
    from concourse.bass_utils import run_bass_kernel_spmd

    x, wq, bq, wk, bk, wv, bv, wo, bo = (
        np.asarray(a, np.float32) for a in (x, wq, bq, wk, bk, wv, bv, wo, bo))

    if "nc" not in _cache:
        _cache["nc"] = _build()
    nc = _cache["nc"]

    in_maps = _host_inputs(x, wq, bq, wk, bk, wv, bv, wo, bo)
    res = run_bass_kernel_spmd(nc, in_maps, core_ids=list(range(NCORES)))
    _cache["last_results"] = res

    const = (bo + bv @ wo.T).astype(np.float32)        # [1024]
    out = np.empty((B, N, E), np.float32)
    for b in range(B):
        out[b] = res.results[2 * b]["out"] + res.results[2 * b + 1]["out"] + const
    return out
